# revision 48
# baseline (speedup 1.0000x reference)
"""NNCLR allswap loss kernel for 8 Trainium2 NeuronCores.

Math (from the reference):
  p = l2norm(projected)  [B=2048, Vg=2, D=256]
  q = l2norm(predicted)  [B=2048, Vt=4, D=256]
  logits[i,j] = p[:,i] @ q[:,j].T / T           (T = 0.2)
  L[i,j] = mean_b( logsumexp_c(logits[i,j,b,:]) - logits[i,j,b,b] )
  Only L[:, :2] is used (Vl = 2), so predicted views 2,3 never touch
  the device.

Sharding: core (rb, j) owns the rb-th 512-row shard and predicted view
j, computing the COMPLETE row sums sum_c exp(logits[i,j,b,c]) over all
2048 columns for i = 0,1.  Because each [128, 2048] PSUM block holds
exactly one (i,j) pair, the ACT accumulator alone produces the finished
row sum - no separate reduction engine, no cross-core combine.

Device work is the irreducible heavy part: 64 fp8 matmuls (537 MMAC)
and 2.1M exp evaluations per core.  Everything cheap-and-serial lives
on the host: l2 normalization is folded into the input marshalling (a
per-row scale during the transpose/fp8 cast; final error 1.6e-5), the
B*4 diagonal dot products, and the final log/mean over 8 KB of sums.

Per-core pipeline (steady state is ACT-bound at ~2.1us per (i,m)):
  PE   : 8 matmuls  [128d x 128b] @ [128d x 512c] -> PSUM [128, 2048]
  ACT  : one FD=2048 ACTIVATE Exp(scale=5) in-place + accum -> row sums
PSUM is two [128, 2048] ping-pong tiles (4 banks each).  Garbage
warmup matmuls during the input DMA keep the PE busy so the HAM clock
gate is fully ramped when real data lands.  pT and qT ship as one
combined 768KB fp8 transfer (single issue, single completion
semaphore); the exp table is prefetched via a dummy activation.
"""

import numpy as np

B = 2048
D = 256
NI = 2            # projected views
NJ = 2            # used predicted views
T = 0.2
RB = 4            # batch-row shards
BL = B // RB      # 512 rows per core
MT = BL // 128    # 4 row m-tiles

_CACHE = {}


def _patch_tile_drain():
    """This walrus build only accepts 1 sync-wait on a Drain (CTRL_NO)
    instruction, but TileContext's tail drain accumulates one wait per
    active processor.  Split the waits across multiple drains."""
    import concourse.tile as tile
    from concourse.vector_clock import ScopedClock

    if getattr(tile.TileContext, "_drain_split_patch", False):
        return

    def _drain_and_barrier(self, tick_clock, wait_clock):
        nc = self.nc
        drain_inst = nc.sync.drain()
        wait_clock.add_sem_waits(
            drain_inst.ins, ScopedClock({None: tick_clock.global_clock})
        )
        si = drain_inst.ins.sync_info
        if si is not None and si.on_wait and len(si.on_wait) > 1:
            waits = list(si.on_wait)
            si.on_wait = waits[:1]
            for w in waits[1:]:
                extra = nc.sync.drain()
                esi = extra.ins.sync_info
                if esi is None:
                    import concourse.mybir as mybir
                    extra.ins.sync_info = mybir.SyncInfo(on_wait=[w], on_update=[])
                else:
                    esi.on_wait = [w]

        nc.all_engine_barrier()
        assert self.sems is not None
        popped = nc._tile_sem_poison_stack.pop()
        assert popped is self._sem_poison
        nc.clear_and_free_semaphores(list(self.sems.allocated().values()))

    tile.TileContext._drain_and_barrier = _drain_and_barrier
    tile.TileContext._drain_split_patch = True


def _split_multiwait(nc, mybir):
    """This walrus build rejects instructions carrying more than one
    semaphore wait.  Hoist excess waits onto standalone EventSemaphore
    instructions inserted just before the original (same engine, in-order
    execution => semantics preserved)."""
    import orjson

    js = orjson.loads(mybir.module_to_json_bytes(nc.m))

    # Delete the Bass-init const-AP memsets and the init all-engine
    # barrier if present (no instruction references the const tiles).
    bb0 = js["functions"][0]["blocks"][0]
    insts = bb0["instructions"]
    ms_idx = [n for n, i in enumerate(insts)
              if i["opcode"] == "Memset"
              and str(i.get("outs", [{}])[0]).find("const-") >= 0]
    if ms_idx:
        lo, hi = ms_idx[0], ms_idx[-1] + 1
        while hi < len(insts) and insts[hi]["opcode"] in ("Drain",
                                                          "EventSemaphore"):
            hi += 1
        bb0["instructions"] = insts[:lo] + insts[hi:]

    ctr = 0
    for f in js["functions"]:
        for bb in f["blocks"]:
            new_insts = []
            for inst in bb["instructions"]:
                si = inst.get("sync_info")
                if si and si.get("on_wait") and len(si["on_wait"]) > 1:
                    waits = si["on_wait"]
                    for w in waits[:-1]:
                        ctr += 1
                        ev = {
                            "engine": inst["engine"],
                            "ins": [],
                            "name": f"WSPLIT-{ctr}",
                            "opcode": "EventSemaphore",
                            "outs": [],
                            "sync_info": {"on_update": [], "on_wait": [w]},
                        }
                        if "debug" in inst:
                            ev["debug"] = inst["debug"]
                        new_insts.append(ev)
                    si["on_wait"] = waits[-1:]
                new_insts.append(inst)
            bb["instructions"] = new_insts
    nc.m = mybir.module_from_json_bytes(orjson.dumps(js))
    return ctr


def _build_program():
    import concourse.bass as bass
    import concourse.tile as tile
    from concourse import mybir
    from contextlib import ExitStack

    _patch_tile_drain()

    fp32 = mybir.dt.float32
    fp8 = mybir.dt.float8e4
    bf16 = mybir.dt.bfloat16
    Exp = mybir.ActivationFunctionType.Exp
    add = mybir.AluOpType.add
    X = mybir.AxisListType.X

    nc = bass.Bass()

    # input: pre-normalized, transposed, fp8-e4m3 (host-marshalled), one
    # combined tensor: cols [0, 2048) = pT [dp, (k, i, b)] with d = k*128+dp,
    # cols [2048, 6144) = qT [dp, (cc2, k, c)] (column half cc2, 1024 each)
    in_all = nc.dram_tensor("in_all", [128, 6144], fp8, kind="ExternalInput")
    outs_t = nc.dram_tensor("outs", [128, 10], fp32, kind="ExternalOutput")
    QO = 2 * NI * BL                                         # qT col offset

    with tile.TileContext(nc) as tc, ExitStack() as ctx:
        res = ctx.enter_context(tc.tile_pool(name="res", bufs=1))
        psum = ctx.enter_context(tc.tile_pool(name="psum", bufs=2, space="PSUM"))

        inp = res.tile([128, 6144], fp8, tag="inp")
        warm_src = res.tile([128, 512], bf16, tag="warm_src")  # never written
        zb = res.tile([128, 1], fp32, tag="zb")
        junk = res.tile([128, 1], fp32, tag="junk")
        stats = res.tile([128, 10], fp32, tag="stats")

        # exp table prefetch chain: memset the bias, then a dummy exp so
        # ACT_TABLE_LOAD runs during the input DMA.
        nc.vector.memset(zb[:], 0.0)
        nc.vector.memset(warm_src[:], 0.0)
        nc.scalar.activation(out=junk[:], in_=zb[:], func=Exp, bias=zb[:])

        # single input DMA: one issue, one completion semaphore
        nc.sync.dma_start(out=inp[:], in_=in_all[:])

        # PE warmup during the DMA: garbage matmuls on never-DMA'd SBUF
        # ramp the HAM clock gate so the first real matmuls run at full
        # clock.  Results are overwritten by later start=True matmuls.
        warm = psum.tile([128, 4, 512], fp32, tag="ps", name="warm")
        for w in range(10):
            nc.tensor.matmul(
                warm[:, w % 4, :],
                lhsT=warm_src[:, 0:128], rhs=warm_src[:, 0:512],
                start=True, stop=True,
            )

        for t in range(NI * MT):
            i, m = divmod(t, MT)
            pst = psum.tile([128, 4, 512], fp32, tag="ps", name=f"ps{t}")
            lhs = [inp[:, k * NI * BL + i * BL + m * 128:
                       k * NI * BL + i * BL + (m + 1) * 128]
                   for k in range(2)]
            for cc2 in range(2):
                for k in range(2):
                    for h in range(2):
                        qcol = QO + cc2 * 2048 + k * 1024 + h * 512
                        nc.tensor.matmul(
                            pst[:, cc2 * 2 + h, :],
                            lhsT=lhs[k],
                            rhs=inp[:, qcol: qcol + 512],
                            start=(k == 0), stop=(k == 1),
                        )
            # exp + complete row sum of one (i,j) block in one ACTIVATE
            nc.scalar.activation(
                out=pst[:],
                in_=pst[:],
                func=Exp, scale=1.0 / T, bias=zb[:],
                accum_out=stats[:, t: t + 1],
            )

        nc.sync.dma_start(out=outs_t[:], in_=stats[:])

    _split_multiwait(nc, mybir)
    return nc


def _get_program():
    if "nc" not in _CACHE:
        _CACHE["nc"] = _build_program()
    return _CACHE["nc"]


def _marshal(projected, predicted):
    import ml_dtypes

    p = np.asarray(projected, dtype=np.float32)            # [B, 2, 256]
    q = np.asarray(predicted, dtype=np.float32)[:, :NJ, :]
    pn = p / np.maximum(np.linalg.norm(p, axis=-1, keepdims=True), 1e-12)
    qn = q / np.maximum(np.linalg.norm(q, axis=-1, keepdims=True), 1e-12)
    pn_c = pn.astype(ml_dtypes.float8_e4m3)
    qn_c = qn.astype(ml_dtypes.float8_e4m3)

    # qT [dp, cc2, k, c] per j (same for all rb)
    qTs = []
    for j in range(NJ):
        qs = qn_c[:, j, :]                                 # [2048, 256]
        a2 = qs.T.reshape(2, 128, 2, 1024)                 # [k, dp, cc2, c]
        qTs.append(np.ascontiguousarray(a2.transpose(1, 2, 0, 3)).reshape(128, -1))

    in_maps = []
    for rb in range(RB):
        ps = pn_c[rb * BL:(rb + 1) * BL]                   # [512, 2, 256]
        arr = ps.transpose(2, 1, 0).reshape(2, 128, NI, BL)      # [k, dp, i, b]
        pT = np.ascontiguousarray(arr.transpose(1, 0, 2, 3)).reshape(128, -1)
        for j in range(NJ):
            in_maps.append(
                {"in_all": np.ascontiguousarray(
                    np.concatenate([pT, qTs[j]], axis=1))})
    # diagonal logits from the same fp8 values the device multiplies
    diag = np.einsum("bid,bjd->ijb", pn_c.astype(np.float64),
                     qn_c.astype(np.float64)) / T          # [NI, NJ, B]
    return in_maps, diag


def kernel(projected, predicted, _trace=False):
    from concourse.bass_utils import run_bass_kernel_spmd

    nc = _get_program()
    in_maps, diag = _marshal(projected, predicted)
    out = run_bass_kernel_spmd(nc, in_maps, list(range(RB * NJ)), trace=_trace)
    results = out.results
    if _trace:
        _CACHE["last_bkr"] = out

    # ---- host combine (float64 for the tiny reductions) ----
    S = np.zeros((NI, NJ, B), dtype=np.float64)
    for rb in range(RB):
        for j in range(NJ):
            es = results[rb * NJ + j]["outs"].astype(np.float64)  # [128, 10]
            for t in range(NI * MT):
                i, m = divmod(t, MT)
                rows = slice(rb * BL + m * 128, rb * BL + (m + 1) * 128)
                S[i, j, rows] = es[:, t]

    lse = np.log(S)
    L = np.mean(lse - diag, axis=-1)          # [NI, NJ]

    global_sum = L[0, 1] + L[1, 0]
    num_global = NI * (NI - 1)
    local_sum = L[0, 0] + L[0, 1] + L[1, 0] + L[1, 1]
    num_local = NI * NJ
    global_loss = global_sum / num_global
    local_loss = local_sum / num_local
    total = (global_sum + local_sum) / (num_global + num_local)
    return np.array([total, global_loss, local_loss], dtype=np.float32)
